# revision 1
# baseline (speedup 1.0000x reference)
"""Trainium2 Bass kernel for fused LoRA linear with per-sequence adapter routing.

Problem (hardcoded shapes):
  x [8192, 4096] fp32, base_weight [4096, 4096], a_cache/b_cache [512, 4096],
  16 sequences x 512 tokens, 8 adapters (rank <= 64), out [8192, 4096]:
      out = x @ base_weight.T + scaling[a(t)] * (x @ A[a(t)].T masked) @ B[a(t)]

Sharding: data-parallel over tokens. Core c handles sequences {2c, 2c+1}
(tokens [1024c, 1024c+1024)) and computes the full 4096 output features for
its tokens. Host-side prep gathers/masks/scales the per-sequence LoRA weights
(tiny) and transposes x/base_weight; all heavy matmuls run on device.

Matmul dtype: float32r (fp32 bits fed to the PE in replicated mode) - 4x the
fp32 matmul rate at ~1.5e-4 relative error (vs ~2.4e-3 for bf16).
"""
import numpy as np

import concourse.bass as bass
import concourse.mybir as mybir
from concourse.bass_utils import run_bass_kernel_spmd

P = 128
NCORES = 8
T_CORE = 1024            # tokens per core (2 sequences)
K = 4096                 # in features
N = 4096                 # out features
KT = K // P              # 32 k-tiles
NCHUNK = 512             # psum free dim per matmul
NC_N = N // NCHUNK       # 8 n-chunks
TT = T_CORE // P         # 8 t-tiles per core
SEQ_LEN = 512
MAX_RANK = 64
WRING = 8                # W streaming ring slots

F32 = mybir.dt.float32
F32R = mybir.dt.float32r

_PROGRAM = None  # cached (nc,) build


def _build_program():
    nc = bass.Bass()
    xT_d = nc.dram_tensor("xT", [K, T_CORE], F32, kind="ExternalInput")
    wt_d = nc.dram_tensor("wt", [K, N], F32, kind="ExternalInput")
    at_d = nc.dram_tensor("at", [K, P], F32, kind="ExternalInput")
    bs_d = nc.dram_tensor("bs", [P, N], F32, kind="ExternalInput")
    out_d = nc.dram_tensor("out", [T_CORE, N], F32, kind="ExternalOutput")

    from contextlib import ExitStack
    with ExitStack() as ctx:
        e = ctx.enter_context
        xT_s = e(nc.sbuf_tensor("xT_s", [P, KT * T_CORE], F32R))     # 128 KB/part
        wt_s = e(nc.sbuf_tensor("wt_s", [P, WRING * NCHUNK], F32R))  # 16 KB/part
        at_s = e(nc.sbuf_tensor("at_s", [P, KT * P], F32R))          # 16 KB/part
        bs_s = e(nc.sbuf_tensor("bs_s", [P, N], F32R))               # 16 KB/part
        xaT_s = e(nc.sbuf_tensor("xaT_s", [P, T_CORE], F32R))         # 4 KB/part
        os_s = e(nc.sbuf_tensor("os_s", [P, TT * NCHUNK], F32))      # 16 KB/part
        banks = [e(nc.psum_tensor(f"pbank{i}", [P, NCHUNK], F32)) for i in range(8)]
        s_ld = e(nc.semaphore("s_ld"))
        s_bs = e(nc.semaphore("s_bs"))
        w_sems = [e(nc.semaphore(f"s_w{i}")) for i in range(WRING)]
        xt_sems = [e(nc.semaphore(f"s_xt{i}")) for i in range(8)]  # 4 xT tiles each
        s_wfree = e(nc.semaphore("s_wfree"))
        s_pexa = e(nc.semaphore("s_pexa"))
        s_zero = e(nc.semaphore("s_zero"))
        s_xa = e(nc.semaphore("s_xa"))
        s_cp = e(nc.semaphore("s_cp"))
        s_bank = e(nc.semaphore("s_bank"))
        od_sems = [e(nc.semaphore(f"s_od{i}")) for i in range(TT)]
        block = e(nc.Block())

        NLD = KT  # at tiles on s_ld; xT on group sems; bs on s_bs

        def wslot(idx):
            return idx % WRING, idx // WRING

        @block.sync
        def _(sync):
            # Phase loads: at tiles first, then xT tiles, then bs.
            for k in range(KT):
                sync.dma_start(
                    out=at_s[:, k * P:(k + 1) * P],
                    in_=at_d[k * P:(k + 1) * P, :].bitcast(F32R),
                ).then_inc(s_ld, 16)
            for k in range(KT):
                sync.dma_start(
                    out=xT_s[:, k * T_CORE:(k + 1) * T_CORE],
                    in_=xT_d[k * P:(k + 1) * P, :].bitcast(F32R),
                ).then_inc(xt_sems[k // 4], 16)
            sync.dma_start(out=bs_s[:], in_=bs_d[:].bitcast(F32R)).then_inc(s_bs, 16)

            # W stream + previous chunk's out stream, interleaved per chunk.
            for c in range(NC_N):
                for k in range(KT):
                    idx = c * KT + k
                    r, rnd = wslot(idx)
                    if idx >= WRING:
                        t = idx - WRING
                        tc_, tk = t // KT, t % KT
                        if tk < KT - 1:
                            sync.wait_ge(s_wfree, tc_ * (KT - 1) + tk + 1)
                        else:
                            sync.wait_ge(s_cp, (tc_ + 1) * TT)
                    sync.dma_start(
                        out=wt_s[:, r * NCHUNK:(r + 1) * NCHUNK],
                        in_=wt_d[k * P:(k + 1) * P, c * NCHUNK:(c + 1) * NCHUNK].bitcast(F32R),
                    ).then_inc(w_sems[r], 16)
                if c >= 1:
                    cc = c - 1
                    for j in range(TT):
                        sync.wait_ge(s_cp, cc * TT + j + 1)
                        sync.dma_start(
                            out=out_d[j * P:(j + 1) * P, cc * NCHUNK:(cc + 1) * NCHUNK],
                            in_=os_s[:, j * NCHUNK:(j + 1) * NCHUNK],
                        ).then_inc(od_sems[j], 16)
            cc = NC_N - 1
            for j in range(TT):
                sync.wait_ge(s_cp, cc * TT + j + 1)
                sync.dma_start(
                    out=out_d[j * P:(j + 1) * P, cc * NCHUNK:(cc + 1) * NCHUNK],
                    in_=os_s[:, j * NCHUNK:(j + 1) * NCHUNK],
                ).then_inc(od_sems[j], 16)

        @block.gpsimd
        def _(gpsimd):
            gpsimd.memset(xaT_s[:].bitcast(F32), 0.0).then_inc(s_zero, 1)

        @block.tensor
        def _(tensor):
            # ---- xa phase ----
            # xaT_full[r, t]; seq0 valid rows 0:64 (t 0:512), seq1 rows 64:128
            # (t 512:1024). psum bank0 <- at.T @ xT[:, 0:512], bank1 <- ... [512:1024].
            tensor.wait_ge(s_ld, 16 * KT)  # all at tiles (full count = safe)
            for k in range(KT):
                if k % 4 == 0:
                    tensor.wait_ge(xt_sems[k // 4], 16 * 4)
                a_sl = at_s[:, k * P:(k + 1) * P]
                m0 = tensor.matmul(
                    banks[0][:], lhsT=a_sl,
                    rhs=xT_s[:, k * T_CORE: k * T_CORE + SEQ_LEN],
                    start=(k == 0), stop=(k == KT - 1))
                m1 = tensor.matmul(
                    banks[1][:], lhsT=a_sl,
                    rhs=xT_s[:, k * T_CORE + SEQ_LEN:(k + 1) * T_CORE],
                    start=(k == 0), stop=(k == KT - 1))
            m0.then_inc(s_pexa, 1)
            m1.then_inc(s_pexa, 1)

            # ---- main loop ----
            tensor.wait_ge(s_bs, 16)         # bs resident
            for g in range(8):
                tensor.wait_ge(xt_sems[g], 16 * 4)
            tensor.wait_ge(s_xa, 2)          # xaT ready (incl. zero pad)
            for c in range(NC_N):
                # lora matmuls open each bank's accumulation group
                for j in range(TT):
                    if c >= 1:
                        tensor.wait_ge(s_cp, (c - 1) * TT + j + 1)
                    tensor.matmul(
                        banks[j][:],
                        lhsT=xaT_s[:, j * P:(j + 1) * P],
                        rhs=bs_s[:, c * NCHUNK:(c + 1) * NCHUNK],
                        start=True, stop=False)
                for k in range(KT):
                    idx = c * KT + k
                    r, rnd = wslot(idx)
                    tensor.wait_ge(w_sems[r], 16 * (rnd + 1))
                    w_sl = wt_s[:, r * NCHUNK:(r + 1) * NCHUNK]
                    for j in range(TT):
                        mm = tensor.matmul(
                            banks[j][:],
                            lhsT=xT_s[:, k * T_CORE + j * P: k * T_CORE + (j + 1) * P],
                            rhs=w_sl,
                            start=False, stop=(k == KT - 1))
                        if k == KT - 1:
                            mm.then_inc(s_bank, 1)
                    if k < KT - 1:
                        # j=7 completion implies j=0..6 complete (pc-order)
                        mm.then_inc(s_wfree, 1)
                    # k=KT-1 slot release comes from the DVE bank-7 copy

        @block.vector
        def _(vector):
            # xa copies into zeroed xaT
            vector.wait_ge(s_zero, 1)
            vector.wait_ge(s_pexa, 2)
            vector.tensor_copy(xaT_s[0:MAX_RANK, 0:SEQ_LEN],
                               banks[0][0:MAX_RANK, :]).then_inc(s_xa, 1)
            vector.tensor_copy(xaT_s[MAX_RANK:P, SEQ_LEN:T_CORE],
                               banks[1][MAX_RANK:P, :]).then_inc(s_xa, 1)
            # out copies psum -> staging
            for c in range(NC_N):
                for j in range(TT):
                    vector.wait_ge(s_bank, c * TT + j + 1)
                    if c >= 1:
                        vector.wait_ge(od_sems[j], 16 * c)
                    vector.tensor_copy(os_s[:, j * NCHUNK:(j + 1) * NCHUNK],
                                        banks[j][:]).then_inc(s_cp, 1)

    return nc


def _get_program():
    global _PROGRAM
    if _PROGRAM is None:
        _PROGRAM = _build_program()
    return _PROGRAM


def _host_prep(x, a_cache, b_cache, base_weight, scaling,
               q_start_loc, q_seqlens, adapter_ids, rank_offset, ranks):
    """Build the 8 per-core input maps (sharding + tiny routing gathers)."""
    x = np.asarray(x, np.float32)
    a_cache = np.asarray(a_cache, np.float32)
    b_cache = np.asarray(b_cache, np.float32)
    base_weight = np.asarray(base_weight, np.float32)
    scaling = np.asarray(scaling, np.float32)
    q_start_loc = np.asarray(q_start_loc, np.int64)
    q_seqlens = np.asarray(q_seqlens, np.int64)
    adapter_ids = np.asarray(adapter_ids, np.int64)
    rank_offset = np.asarray(rank_offset, np.int64)
    ranks = np.asarray(ranks, np.int64)

    T = x.shape[0]
    assert T == NCORES * T_CORE
    # exact reference routing: per-token adapter, then check 512-block uniformity
    tok = np.arange(T)
    seq_idx = np.searchsorted(q_start_loc, tok, side="right") - 1
    tok_adapter = adapter_ids[seq_idx]
    blocks = tok_adapter.reshape(T // SEQ_LEN, SEQ_LEN)
    assert (blocks == blocks[:, :1]).all(), "non-uniform 512-token blocks"
    block_adapter = blocks[:, 0]  # [16]

    wt = np.ascontiguousarray(base_weight.T)  # [K, N]

    in_maps = []
    for c in range(NCORES):
        rows = slice(c * T_CORE, (c + 1) * T_CORE)
        xT = np.ascontiguousarray(x[rows].T)  # [K, T_CORE]
        at = np.zeros((K, P), np.float32)
        bs = np.zeros((P, N), np.float32)
        for s in range(2):  # two sequences per core
            a = int(block_adapter[2 * c + s])
            r = int(ranks[a])
            idxs = rank_offset[a, :r]
            at[:, s * MAX_RANK: s * MAX_RANK + r] = a_cache[idxs].T
            bs[s * MAX_RANK: s * MAX_RANK + r, :] = b_cache[idxs] * scaling[a]
        in_maps.append({"xT": xT, "wt": wt, "at": at, "bs": bs})
    return in_maps


LAST_RESULT = None  # BassKernelResults of the most recent run (for profiling)


def kernel(**inputs) -> np.ndarray:
    global LAST_RESULT
    import os
    nc = _get_program()
    in_maps = _host_prep(**inputs)
    trace = os.environ.get("KERNEL_TRACE") == "1"
    kw = {}
    if trace:
        kw = dict(trace=True, trace_cores=list(range(NCORES)))
    res = run_bass_kernel_spmd(nc, in_maps, core_ids=list(range(NCORES)), **kw)
    LAST_RESULT = res
    return np.concatenate([res.results[c]["out"] for c in range(NCORES)], axis=0)



# revision 2
# speedup vs baseline: 1.1105x; 1.1105x over previous
"""Trainium2 Bass kernel for fused LoRA linear with per-sequence adapter routing.

Problem (hardcoded shapes):
  x [8192, 4096] fp32, base_weight [4096, 4096], a_cache/b_cache [512, 4096],
  16 sequences x 512 tokens, 8 adapters (rank <= 64), out [8192, 4096]:
      out = x @ base_weight.T + scaling[a(t)] * (x @ A[a(t)].T masked) @ B[a(t)]

Sharding: data-parallel over tokens. Core c handles sequences {2c, 2c+1}
(tokens [1024c, 1024c+1024)) and computes the full 4096 output features for
its tokens. Host-side prep gathers/masks/scales the per-sequence LoRA weights
(tiny), converts x/W/A/B to bf16 and lays every DRAM tensor out in its exact
SBUF layout so each DMA is one large fully-contiguous transfer.

Device schedule (per core):
  - SP queue: at (1 DMA), xT (8 DMAs of 4 k-tiles), bs (1 DMA).
  - Activation queue: w chunk stream (8 DMAs of 4 MB, 2-slot SBUF ring)
    interleaved with the per-chunk output stores.
  - PE: xa = A.T @ xT interleaved with xT arrival; chunk 0 base matmuls for
    t-tiles 2..7 interleaved as well (LoRA applied as the *last* accumulant
    for chunk 0, first for chunks 1..7); then chunks 1..7 steady-state.
  - DVE: PSUM -> SBUF copies (xa with fp32->bf16 cast, outputs fp32).

All matmuls bf16 (1 cycle/row, fast weight load), fp32 PSUM accumulation.
"""
import numpy as np

import concourse.bass as bass
import concourse.mybir as mybir
from concourse.bass_utils import run_bass_kernel_spmd

P = 128
NCORES = 8
T_CORE = 1024            # tokens per core (2 sequences)
K = 4096                 # in features
N = 4096                 # out features
KT = K // P              # 32 k-tiles
NCHUNK = 512             # psum free dim per matmul
NC_N = N // NCHUNK       # 8 n-chunks
TT = T_CORE // P         # 8 t-tiles per core
SEQ_LEN = 512
MAX_RANK = 64
KG = 4                   # k-tiles per xT DMA group
NG = KT // KG            # 8 xT groups
WSLOT = KT * NCHUNK      # w ring slot width (one n-chunk, all k)

F32 = mybir.dt.float32
BF16 = mybir.dt.bfloat16
NP_BF16 = mybir.dt.np(BF16)

_PROGRAM = None  # cached (nc,) build


def _build_program():
    nc = bass.Bass()
    xt_d = nc.dram_tensor("xt", [P, KT * T_CORE], BF16, kind="ExternalInput")
    wt_d = nc.dram_tensor("wt", [P, NC_N * WSLOT], BF16, kind="ExternalInput")
    at_d = nc.dram_tensor("at", [P, KT * P], BF16, kind="ExternalInput")
    bs_d = nc.dram_tensor("bs", [P, N], BF16, kind="ExternalInput")
    out_d = nc.dram_tensor("out", [P, NC_N * TT * NCHUNK], F32, kind="ExternalOutput")

    from contextlib import ExitStack
    with ExitStack() as ctx:
        e = ctx.enter_context
        xT_s = e(nc.sbuf_tensor("xT_s", [P, KT * T_CORE], BF16))   # 64 KB/part
        w_s = e(nc.sbuf_tensor("w_s", [P, 2 * WSLOT], BF16))       # 64 KB/part
        at_s = e(nc.sbuf_tensor("at_s", [P, KT * P], BF16))        # 8 KB/part
        bs_s = e(nc.sbuf_tensor("bs_s", [P, N], BF16))             # 8 KB/part
        xaT_s = e(nc.sbuf_tensor("xaT_s", [P, T_CORE], BF16))      # 2 KB/part
        os_s = e(nc.sbuf_tensor("os_s", [P, TT * NCHUNK], F32))    # 16 KB/part
        banks = [e(nc.psum_tensor(f"pbank{i}", [P, NCHUNK], F32)) for i in range(8)]
        s_at = e(nc.semaphore("s_at"))
        xt_sems = [e(nc.semaphore(f"s_xt{i}")) for i in range(NG)]
        s_bs = e(nc.semaphore("s_bs"))
        w_sems = [e(nc.semaphore(f"s_w{i}")) for i in range(2)]
        s_pexa = e(nc.semaphore("s_pexa"))
        s_zero = e(nc.semaphore("s_zero"))
        s_xa = e(nc.semaphore("s_xa"))
        s_bank = e(nc.semaphore("s_bank"))
        s_cp = e(nc.semaphore("s_cp"))
        s_od = e(nc.semaphore("s_od"))
        block = e(nc.Block())

        def xts(k, lo, hi):
            return xT_s[:, k * T_CORE + lo:k * T_CORE + hi]

        def xtile(k, j):
            return xT_s[:, k * T_CORE + j * P:k * T_CORE + (j + 1) * P]

        def wsl(c, k):
            base = (c % 2) * WSLOT + k * NCHUNK
            return w_s[:, base:base + NCHUNK]

        @block.sync
        def _(sync):
            sync.dma_start(out=at_s[:], in_=at_d[:]).then_inc(s_at, 16)
            gw = KG * T_CORE
            for g in range(NG):
                sync.dma_start(
                    out=xT_s[:, g * gw:(g + 1) * gw],
                    in_=xt_d[:, g * gw:(g + 1) * gw],
                ).then_inc(xt_sems[g], 16)
            sync.dma_start(out=bs_s[:], in_=bs_d[:]).then_inc(s_bs, 16)

        @block.scalar
        def _(scalar):
            # w chunk stream (2-slot ring) interleaved with output stores.
            ow = TT * NCHUNK
            for c in range(NC_N):
                if c >= 2:
                    # slot's previous occupant (chunk c-2) fully consumed once
                    # its PSUM copies are done
                    scalar.wait_ge(s_cp, (c - 1) * TT)
                scalar.dma_start(
                    out=w_s[:, (c % 2) * WSLOT:(c % 2 + 1) * WSLOT],
                    in_=wt_d[:, c * WSLOT:(c + 1) * WSLOT],
                ).then_inc(w_sems[c % 2], 16)
                if c >= 2:
                    cc = c - 2
                    scalar.wait_ge(s_cp, (cc + 1) * TT)
                    scalar.dma_start(
                        out=out_d[:, cc * ow:(cc + 1) * ow], in_=os_s[:],
                    ).then_inc(s_od, 16)
            for cc in (NC_N - 2, NC_N - 1):
                scalar.wait_ge(s_cp, (cc + 1) * TT)
                scalar.dma_start(
                    out=out_d[:, cc * ow:(cc + 1) * ow], in_=os_s[:],
                ).then_inc(s_od, 16)

        @block.gpsimd
        def _(gpsimd):
            gpsimd.memset(xaT_s[:], 0.0).then_inc(s_zero, 1)

        @block.tensor
        def _(tensor):
            # ---- xa phase + chunk-0 base (t-tiles 2..7), interleaved with
            # xT arrival ----
            tensor.wait_ge(s_at, 16)
            for g in range(NG):
                tensor.wait_ge(xt_sems[g], 16)
                for k in range(g * KG, (g + 1) * KG):
                    a_sl = at_s[:, k * P:(k + 1) * P]
                    m0 = tensor.matmul(
                        banks[0][:], lhsT=a_sl, rhs=xts(k, 0, SEQ_LEN),
                        start=(k == 0), stop=(k == KT - 1))
                    m1 = tensor.matmul(
                        banks[1][:], lhsT=a_sl, rhs=xts(k, SEQ_LEN, T_CORE),
                        start=(k == 0), stop=(k == KT - 1))
                if g == 0:
                    tensor.wait_ge(w_sems[0], 16)
                for k in range(g * KG, (g + 1) * KG):
                    for j in range(2, TT):
                        tensor.matmul(
                            banks[j][:], lhsT=xtile(k, j), rhs=wsl(0, k),
                            start=(k == 0), stop=False)
            m0.then_inc(s_pexa, 1)
            m1.then_inc(s_pexa, 1)

            # chunk 0, t-tiles 0..1 (banks freed by the xaT copies)
            tensor.wait_ge(s_xa, 2)
            for j in range(2):
                for k in range(KT):
                    tensor.matmul(
                        banks[j][:], lhsT=xtile(k, j), rhs=wsl(0, k),
                        start=(k == 0), stop=False)
            # chunk 0 lora (last accumulant), closes each bank
            tensor.wait_ge(s_bs, 16)
            for j in range(TT):
                tensor.matmul(
                    banks[j][:], lhsT=xaT_s[:, j * P:(j + 1) * P],
                    rhs=bs_s[:, 0:NCHUNK], start=False, stop=True,
                ).then_inc(s_bank, 1)

            # ---- chunks 1..7 steady state (lora first, then k-loop) ----
            for c in range(1, NC_N):
                tensor.wait_ge(w_sems[c % 2], 16 * (c // 2 + 1))
                for j in range(TT):
                    tensor.wait_ge(s_cp, (c - 1) * TT + j + 1)
                    tensor.matmul(
                        banks[j][:], lhsT=xaT_s[:, j * P:(j + 1) * P],
                        rhs=bs_s[:, c * NCHUNK:(c + 1) * NCHUNK],
                        start=True, stop=False)
                for k in range(KT):
                    for j in range(TT):
                        mm = tensor.matmul(
                            banks[j][:], lhsT=xtile(k, j), rhs=wsl(c, k),
                            start=False, stop=(k == KT - 1))
                        if k == KT - 1:
                            mm.then_inc(s_bank, 1)

        @block.vector
        def _(vector):
            # xa copies into zeroed xaT (fp32 PSUM -> bf16, valid halves only)
            vector.wait_ge(s_zero, 1)
            vector.wait_ge(s_pexa, 2)
            vector.tensor_copy(xaT_s[0:MAX_RANK, 0:SEQ_LEN],
                               banks[0][0:MAX_RANK, :]).then_inc(s_xa, 1)
            vector.tensor_copy(xaT_s[MAX_RANK:P, SEQ_LEN:T_CORE],
                               banks[1][MAX_RANK:P, :]).then_inc(s_xa, 1)
            # out copies psum -> staging
            for c in range(NC_N):
                for j in range(TT):
                    vector.wait_ge(s_bank, c * TT + j + 1)
                    if c >= 1 and j == 0:
                        vector.wait_ge(s_od, 16 * c)  # os_s store done
                    vector.tensor_copy(os_s[:, j * NCHUNK:(j + 1) * NCHUNK],
                                       banks[j][:]).then_inc(s_cp, 1)

    return nc


def _get_program():
    global _PROGRAM
    if _PROGRAM is None:
        _PROGRAM = _build_program()
    return _PROGRAM


def _host_prep(x, a_cache, b_cache, base_weight, scaling,
               q_start_loc, q_seqlens, adapter_ids, rank_offset, ranks):
    """Build the 8 per-core input maps (sharding + tiny routing gathers)."""
    x = np.asarray(x, np.float32)
    a_cache = np.asarray(a_cache, np.float32)
    b_cache = np.asarray(b_cache, np.float32)
    base_weight = np.asarray(base_weight, np.float32)
    scaling = np.asarray(scaling, np.float32)
    q_start_loc = np.asarray(q_start_loc, np.int64)
    adapter_ids = np.asarray(adapter_ids, np.int64)
    rank_offset = np.asarray(rank_offset, np.int64)
    ranks = np.asarray(ranks, np.int64)

    T = x.shape[0]
    assert T == NCORES * T_CORE
    # exact reference routing: per-token adapter, then check 512-block uniformity
    tok = np.arange(T)
    seq_idx = np.searchsorted(q_start_loc, tok, side="right") - 1
    tok_adapter = adapter_ids[seq_idx]
    blocks = tok_adapter.reshape(T // SEQ_LEN, SEQ_LEN)
    assert (blocks == blocks[:, :1]).all(), "non-uniform 512-token blocks"
    block_adapter = blocks[:, 0]  # [16]

    xb = x.astype(NP_BF16)
    # wt layout: wt[p, (c*KT + k)*512 + n] = W[c*512 + n, k*128 + p]
    wb = np.ascontiguousarray(base_weight.T).astype(NP_BF16)  # [K, N]
    wt = np.ascontiguousarray(
        wb.reshape(KT, P, NC_N, NCHUNK).transpose(1, 2, 0, 3)
    ).reshape(P, NC_N * WSLOT)

    in_maps = []
    for c in range(NCORES):
        rows = slice(c * T_CORE, (c + 1) * T_CORE)
        # xt layout: xt[p, k*1024 + t] = x[row0 + t, k*128 + p]
        xt = np.ascontiguousarray(
            xb[rows].T.reshape(KT, P, T_CORE).transpose(1, 0, 2)
        ).reshape(P, KT * T_CORE)
        a_pack = np.zeros((P, K), np.float32)
        bs = np.zeros((P, N), np.float32)
        for s in range(2):  # two sequences per core
            a = int(block_adapter[2 * c + s])
            r = int(ranks[a])
            idxs = rank_offset[a, :r]
            a_pack[s * MAX_RANK: s * MAX_RANK + r, :] = a_cache[idxs]
            bs[s * MAX_RANK: s * MAX_RANK + r, :] = b_cache[idxs] * scaling[a]
        # at layout: at[p, k*128 + r] = a_pack[r, k*128 + p]
        at = np.ascontiguousarray(
            a_pack.T.astype(NP_BF16).reshape(KT, P, P).transpose(1, 0, 2)
        ).reshape(P, KT * P)
        in_maps.append({"xt": xt, "wt": wt, "at": at,
                        "bs": bs.astype(NP_BF16)})
    return in_maps


LAST_RESULT = None  # BassKernelResults of the most recent run (for profiling)


def kernel(**inputs) -> np.ndarray:
    global LAST_RESULT
    import os
    nc = _get_program()
    in_maps = _host_prep(**inputs)
    trace = os.environ.get("KERNEL_TRACE") == "1"
    kw = {}
    if trace:
        kw = dict(trace=True, trace_cores=list(range(NCORES)))
    res = run_bass_kernel_spmd(nc, in_maps, core_ids=list(range(NCORES)), **kw)
    LAST_RESULT = res
    out = np.empty((NCORES * T_CORE, N), np.float32)
    for c in range(NCORES):
        # out buf: [p, (cc*TT + j)*512 + n] -> out[j*128 + p, cc*512 + n]
        buf = res.results[c]["out"].reshape(P, NC_N, TT, NCHUNK)
        out[c * T_CORE:(c + 1) * T_CORE] = (
            buf.transpose(2, 0, 1, 3).reshape(T_CORE, N))
    return out


# revision 3
# speedup vs baseline: 1.2011x; 1.0815x over previous
"""Trainium2 Bass kernel for fused LoRA linear with per-sequence adapter routing.

Problem (hardcoded shapes):
  x [8192, 4096] fp32, base_weight [4096, 4096], a_cache/b_cache [512, 4096],
  16 sequences x 512 tokens, 8 adapters (rank <= 64), out [8192, 4096]:
      out = x @ base_weight.T + scaling[a(t)] * (x @ A[a(t)].T masked) @ B[a(t)]

Sharding: data-parallel over tokens. Core c handles sequences {2c, 2c+1}
(tokens [1024c, 1024c+1024)) and computes the full 4096 output features for
its tokens. Host-side prep gathers/masks/scales the per-sequence LoRA weights
(tiny), converts x/W/A/B to bf16 and lays every DRAM tensor out in its exact
SBUF layout so each DMA is one large fully-contiguous transfer.

Device schedule (per core):
  - SP queue: at (1 DMA), xT (8 DMAs of 4 k-tiles), bs (1 DMA).
  - Activation queue: w chunk stream (2-slot SBUF ring; chunk 0 split in 4
    pieces gated on xT arrival so startup loads aren't starved on the shared
    SDMA engines) interleaved with the per-chunk output stores (2 pieces).
  - PE: xa = A.T @ xT interleaved with xT arrival and with chunk-0 base
    matmuls for t-tiles 2..7 (LoRA applied as the *last* accumulant for
    chunk 0, first for chunks 1..7). Chunks 1..7 run j-outer/k-inner so each
    PSUM bank's stop lands ~7us before the next chunk needs it -> the DVE
    drain copies never stall the PE.
  - DVE: PSUM -> SBUF copies (xa with fp32->bf16 cast, outputs fp32).

All matmuls bf16 (1 cycle/row, fast weight load), fp32 PSUM accumulation.
"""
import numpy as np

import concourse.bass as bass
import concourse.mybir as mybir
from concourse.bass_utils import run_bass_kernel_spmd

P = 128
NCORES = 8
T_CORE = 1024            # tokens per core (2 sequences)
K = 4096                 # in features
N = 4096                 # out features
KT = K // P              # 32 k-tiles
NCHUNK = 512             # psum free dim per matmul
NC_N = N // NCHUNK       # 8 n-chunks
TT = T_CORE // P         # 8 t-tiles per core
SEQ_LEN = 512
MAX_RANK = 64
KG = 4                   # k-tiles per xT DMA group
NG = KT // KG            # 8 xT groups
WSLOT = KT * NCHUNK      # w ring slot width (one n-chunk, all k)
WPIECE = WSLOT // 4      # chunk-0 w DMA piece (8 k-tiles)

F32 = mybir.dt.float32
BF16 = mybir.dt.bfloat16
NP_BF16 = mybir.dt.np(BF16)

_PROGRAM = None  # cached (nc,) build


def _build_program():
    nc = bass.Bass()
    xt_d = nc.dram_tensor("xt", [P, KT * T_CORE], BF16, kind="ExternalInput")
    wt_d = nc.dram_tensor("wt", [P, NC_N * WSLOT], BF16, kind="ExternalInput")
    at_d = nc.dram_tensor("at", [P, KT * P], BF16, kind="ExternalInput")
    bs_d = nc.dram_tensor("bs", [P, N], BF16, kind="ExternalInput")
    out_d = nc.dram_tensor("out", [P, NC_N * TT * NCHUNK], F32, kind="ExternalOutput")

    from contextlib import ExitStack
    with ExitStack() as ctx:
        e = ctx.enter_context
        xT_s = e(nc.sbuf_tensor("xT_s", [P, KT * T_CORE], BF16))   # 64 KB/part
        w_s = e(nc.sbuf_tensor("w_s", [P, 2 * WSLOT], BF16))       # 64 KB/part
        at_s = e(nc.sbuf_tensor("at_s", [P, KT * P], BF16))        # 8 KB/part
        bs_s = e(nc.sbuf_tensor("bs_s", [P, N], BF16))             # 8 KB/part
        xaT_s = e(nc.sbuf_tensor("xaT_s", [P, T_CORE], BF16))      # 2 KB/part
        os_s = e(nc.sbuf_tensor("os_s", [P, TT * NCHUNK], F32))    # 16 KB/part
        banks = [e(nc.psum_tensor(f"pbank{i}", [P, NCHUNK], F32)) for i in range(8)]
        s_at = e(nc.semaphore("s_at"))
        xt_sems = [e(nc.semaphore(f"s_xt{i}")) for i in range(NG)]
        s_bs = e(nc.semaphore("s_bs"))
        w_sems = [e(nc.semaphore(f"s_w{i}")) for i in range(2)]
        s_pexa = e(nc.semaphore("s_pexa"))
        s_zero = e(nc.semaphore("s_zero"))
        s_xa = e(nc.semaphore("s_xa"))
        s_bank = e(nc.semaphore("s_bank"))
        s_cp = e(nc.semaphore("s_cp"))
        s_od = e(nc.semaphore("s_od"))
        block = e(nc.Block())

        def xts(k, lo, hi):
            return xT_s[:, k * T_CORE + lo:k * T_CORE + hi]

        def xtile(k, j):
            return xT_s[:, k * T_CORE + j * P:k * T_CORE + (j + 1) * P]

        def wsl(c, k):
            base = (c % 2) * WSLOT + k * NCHUNK
            return w_s[:, base:base + NCHUNK]

        def wslot_ready(c):
            # w_sems value guaranteeing chunk c resident: slot 0 gets chunk 0
            # in 4 pieces (16 each), then 2,4,6; slot 1 gets 1,3,5,7.
            if c % 2 == 0:
                return 64 + 16 * (c // 2)
            return 16 * ((c + 1) // 2)

        @block.sync
        def _(sync):
            sync.dma_start(out=at_s[:], in_=at_d[:]).then_inc(s_at, 16)
            gw = KG * T_CORE
            for g in range(NG):
                sync.dma_start(
                    out=xT_s[:, g * gw:(g + 1) * gw],
                    in_=xt_d[:, g * gw:(g + 1) * gw],
                ).then_inc(xt_sems[g], 16)
            sync.dma_start(out=bs_s[:], in_=bs_d[:]).then_inc(s_bs, 16)

        @block.scalar
        def _(scalar):
            # w chunk stream (2-slot ring) interleaved with output stores.
            # chunk 0 in 4 pieces, gated so the startup-critical at/xT loads
            # aren't starved on the shared SDMA engines.
            for i in range(4):
                if i > 0:
                    scalar.wait_ge(xt_sems[2 * i - 1], 16)
                scalar.dma_start(
                    out=w_s[:, i * WPIECE:(i + 1) * WPIECE],
                    in_=wt_d[:, i * WPIECE:(i + 1) * WPIECE],
                ).then_inc(w_sems[0], 16)
            scalar.wait_ge(xt_sems[NG - 1], 16)
            ow = TT * NCHUNK
            hw_ = ow // 2

            def store(cc, piece):
                scalar.wait_ge(s_cp, cc * TT + 4 * (piece + 1))
                scalar.dma_start(
                    out=out_d[:, cc * ow + piece * hw_:cc * ow + (piece + 1) * hw_],
                    in_=os_s[:, piece * hw_:(piece + 1) * hw_],
                ).then_inc(s_od, 16)

            for c in range(1, NC_N):
                if c >= 2:
                    # slot's previous occupant (chunk c-2) fully drained
                    scalar.wait_ge(s_cp, (c - 1) * TT)
                scalar.dma_start(
                    out=w_s[:, (c % 2) * WSLOT:(c % 2 + 1) * WSLOT],
                    in_=wt_d[:, c * WSLOT:(c + 1) * WSLOT],
                ).then_inc(w_sems[c % 2], 16)
                if c >= 2:
                    store(c - 2, 0)
                    store(c - 2, 1)
            for cc in (NC_N - 2, NC_N - 1):
                store(cc, 0)
                store(cc, 1)

        @block.gpsimd
        def _(gpsimd):
            gpsimd.memset(xaT_s[:], 0.0).then_inc(s_zero, 1)

        @block.tensor
        def _(tensor):
            # ---- xa phase + chunk-0 base (t-tiles 2..7), interleaved with
            # xT arrival ----
            tensor.wait_ge(s_at, 16)
            for g in range(NG):
                tensor.wait_ge(xt_sems[g], 16)
                for k in range(g * KG, (g + 1) * KG):
                    a_sl = at_s[:, k * P:(k + 1) * P]
                    m0 = tensor.matmul(
                        banks[0][:], lhsT=a_sl, rhs=xts(k, 0, SEQ_LEN),
                        start=(k == 0), stop=(k == KT - 1))
                    m1 = tensor.matmul(
                        banks[1][:], lhsT=a_sl, rhs=xts(k, SEQ_LEN, T_CORE),
                        start=(k == 0), stop=(k == KT - 1))
                tensor.wait_ge(w_sems[0], 16 * (g // 2 + 1))
                for k in range(g * KG, (g + 1) * KG):
                    for j in range(2, TT):
                        tensor.matmul(
                            banks[j][:], lhsT=xtile(k, j), rhs=wsl(0, k),
                            start=(k == 0), stop=False)
            m0.then_inc(s_pexa, 1)
            m1.then_inc(s_pexa, 1)

            # chunk 0, t-tiles 0..1 (banks freed by the xaT copies), then the
            # lora closes (stops j0, j1, j2..j7 in order)
            tensor.wait_ge(s_xa, 1)
            for k in range(KT):
                tensor.matmul(banks[0][:], lhsT=xtile(k, 0), rhs=wsl(0, k),
                              start=(k == 0), stop=False)
            tensor.wait_ge(s_xa, 2)
            tensor.wait_ge(s_bs, 16)
            tensor.matmul(
                banks[0][:], lhsT=xaT_s[:, 0:P], rhs=bs_s[:, 0:NCHUNK],
                start=False, stop=True).then_inc(s_bank, 1)
            for k in range(KT):
                tensor.matmul(banks[1][:], lhsT=xtile(k, 1), rhs=wsl(0, k),
                              start=(k == 0), stop=False)
            tensor.matmul(
                banks[1][:], lhsT=xaT_s[:, P:2 * P], rhs=bs_s[:, 0:NCHUNK],
                start=False, stop=True).then_inc(s_bank, 1)
            for j in range(2, TT):
                tensor.matmul(
                    banks[j][:], lhsT=xaT_s[:, j * P:(j + 1) * P],
                    rhs=bs_s[:, 0:NCHUNK], start=False, stop=True,
                ).then_inc(s_bank, 1)

            # ---- chunks 1..7 steady state: j-outer / k-inner ----
            for c in range(1, NC_N):
                tensor.wait_ge(w_sems[c % 2], wslot_ready(c))
                for j in range(TT):
                    tensor.wait_ge(s_cp, (c - 1) * TT + j + 1)
                    tensor.matmul(
                        banks[j][:], lhsT=xaT_s[:, j * P:(j + 1) * P],
                        rhs=bs_s[:, c * NCHUNK:(c + 1) * NCHUNK],
                        start=True, stop=False)
                    for k in range(KT):
                        mm = tensor.matmul(
                            banks[j][:], lhsT=xtile(k, j), rhs=wsl(c, k),
                            start=False, stop=(k == KT - 1))
                    mm.then_inc(s_bank, 1)

        @block.vector
        def _(vector):
            # xa copies into zeroed xaT (fp32 PSUM -> bf16, valid halves only)
            vector.wait_ge(s_zero, 1)
            vector.wait_ge(s_pexa, 2)
            vector.tensor_copy(xaT_s[0:MAX_RANK, 0:SEQ_LEN],
                               banks[0][0:MAX_RANK, :]).then_inc(s_xa, 1)
            vector.tensor_copy(xaT_s[MAX_RANK:P, SEQ_LEN:T_CORE],
                               banks[1][MAX_RANK:P, :]).then_inc(s_xa, 1)
            # out copies psum -> staging
            for c in range(NC_N):
                for j in range(TT):
                    vector.wait_ge(s_bank, c * TT + j + 1)
                    if c >= 1 and j == 0:
                        vector.wait_ge(s_od, 32 * (c - 1) + 16)
                    if c >= 1 and j == 4:
                        vector.wait_ge(s_od, 32 * (c - 1) + 32)
                    vector.tensor_copy(os_s[:, j * NCHUNK:(j + 1) * NCHUNK],
                                       banks[j][:]).then_inc(s_cp, 1)

    return nc


def _get_program():
    global _PROGRAM
    if _PROGRAM is None:
        _PROGRAM = _build_program()
    return _PROGRAM


def _host_prep(x, a_cache, b_cache, base_weight, scaling,
               q_start_loc, q_seqlens, adapter_ids, rank_offset, ranks):
    """Build the 8 per-core input maps (sharding + tiny routing gathers)."""
    x = np.asarray(x, np.float32)
    a_cache = np.asarray(a_cache, np.float32)
    b_cache = np.asarray(b_cache, np.float32)
    base_weight = np.asarray(base_weight, np.float32)
    scaling = np.asarray(scaling, np.float32)
    q_start_loc = np.asarray(q_start_loc, np.int64)
    adapter_ids = np.asarray(adapter_ids, np.int64)
    rank_offset = np.asarray(rank_offset, np.int64)
    ranks = np.asarray(ranks, np.int64)

    T = x.shape[0]
    assert T == NCORES * T_CORE
    # exact reference routing: per-token adapter, then check 512-block uniformity
    tok = np.arange(T)
    seq_idx = np.searchsorted(q_start_loc, tok, side="right") - 1
    tok_adapter = adapter_ids[seq_idx]
    blocks = tok_adapter.reshape(T // SEQ_LEN, SEQ_LEN)
    assert (blocks == blocks[:, :1]).all(), "non-uniform 512-token blocks"
    block_adapter = blocks[:, 0]  # [16]

    xb = x.astype(NP_BF16)
    # wt layout: wt[p, (c*KT + k)*512 + n] = W[c*512 + n, k*128 + p]
    wb = np.ascontiguousarray(base_weight.T).astype(NP_BF16)  # [K, N]
    wt = np.ascontiguousarray(
        wb.reshape(KT, P, NC_N, NCHUNK).transpose(1, 2, 0, 3)
    ).reshape(P, NC_N * WSLOT)

    in_maps = []
    for c in range(NCORES):
        rows = slice(c * T_CORE, (c + 1) * T_CORE)
        # xt layout: xt[p, k*1024 + t] = x[row0 + t, k*128 + p]
        xt = np.ascontiguousarray(
            xb[rows].T.reshape(KT, P, T_CORE).transpose(1, 0, 2)
        ).reshape(P, KT * T_CORE)
        a_pack = np.zeros((P, K), np.float32)
        bs = np.zeros((P, N), np.float32)
        for s in range(2):  # two sequences per core
            a = int(block_adapter[2 * c + s])
            r = int(ranks[a])
            idxs = rank_offset[a, :r]
            a_pack[s * MAX_RANK: s * MAX_RANK + r, :] = a_cache[idxs]
            bs[s * MAX_RANK: s * MAX_RANK + r, :] = b_cache[idxs] * scaling[a]
        # at layout: at[p, k*128 + r] = a_pack[r, k*128 + p]
        at = np.ascontiguousarray(
            a_pack.T.astype(NP_BF16).reshape(KT, P, P).transpose(1, 0, 2)
        ).reshape(P, KT * P)
        in_maps.append({"xt": xt, "wt": wt, "at": at,
                        "bs": bs.astype(NP_BF16)})
    return in_maps


LAST_RESULT = None  # BassKernelResults of the most recent run (for profiling)


def kernel(**inputs) -> np.ndarray:
    global LAST_RESULT
    import os
    nc = _get_program()
    in_maps = _host_prep(**inputs)
    trace = os.environ.get("KERNEL_TRACE") == "1"
    kw = {}
    if trace:
        kw = dict(trace=True, trace_cores=list(range(NCORES)))
    res = run_bass_kernel_spmd(nc, in_maps, core_ids=list(range(NCORES)), **kw)
    LAST_RESULT = res
    out = np.empty((NCORES * T_CORE, N), np.float32)
    for c in range(NCORES):
        # out buf: [p, (cc*TT + j)*512 + n] -> out[j*128 + p, cc*512 + n]
        buf = res.results[c]["out"].reshape(P, NC_N, TT, NCHUNK)
        out[c * T_CORE:(c + 1) * T_CORE] = (
            buf.transpose(2, 0, 1, 3).reshape(T_CORE, N))
    return out


# revision 6
# speedup vs baseline: 1.2112x; 1.0084x over previous
"""Trainium2 Bass kernel for fused LoRA linear with per-sequence adapter routing.

Problem (hardcoded shapes):
  x [8192, 4096] fp32, base_weight [4096, 4096], a_cache/b_cache [512, 4096],
  16 sequences x 512 tokens, 8 adapters (rank <= 64), out [8192, 4096]:
      out = x @ base_weight.T + scaling[a(t)] * (x @ A[a(t)].T masked) @ B[a(t)]

Sharding: data-parallel over tokens. Core c handles sequences {2c, 2c+1}
(tokens [1024c, 1024c+1024)) and computes the full 4096 output features for
its tokens. Host-side prep gathers/masks/scales the per-sequence LoRA weights
(tiny), converts x/W/A/B to bf16 and lays every DRAM tensor out in its exact
SBUF layout so each DMA is one large fully-contiguous transfer.

Device schedule (per core):
  - SP queue: at (1 DMA), xT (8 DMAs of 4 k-tiles), bs (1 DMA).
  - Activation queue: w chunk stream (2-slot SBUF ring; chunk 0 split in 4
    pieces gated on xT arrival so startup loads aren't starved on the shared
    SDMA engines) interleaved with the per-chunk output stores (2 pieces).
  - PE: xa = A.T @ xT interleaved with xT arrival and with chunk-0 base
    matmuls for t-tiles 2..7 (LoRA applied as the *last* accumulant for
    chunk 0, first for chunks 1..7). Chunks 1..7 run j-outer/k-inner so each
    PSUM bank's stop lands ~7us before the next chunk needs it -> the DVE
    drain copies never stall the PE.
  - DVE: PSUM -> SBUF copies (xa with fp32->bf16 cast, outputs fp32).

All matmuls bf16 (1 cycle/row, fast weight load), fp32 PSUM accumulation.
"""
import numpy as np

import concourse.bass as bass
import concourse.mybir as mybir
from concourse.bass_utils import run_bass_kernel_spmd

P = 128
NCORES = 8
T_CORE = 1024            # tokens per core (2 sequences)
K = 4096                 # in features
N = 4096                 # out features
KT = K // P              # 32 k-tiles
NCHUNK = 512             # psum free dim per matmul
NC_N = N // NCHUNK       # 8 n-chunks
TT = T_CORE // P         # 8 t-tiles per core
SEQ_LEN = 512
MAX_RANK = 64
KG = 4                   # k-tiles per xT DMA group
NG = KT // KG            # 8 xT groups
WSLOT = KT * NCHUNK      # w ring slot width (one n-chunk, all k)
WPIECE = WSLOT // 4      # chunk-0 w DMA piece (8 k-tiles)

F32 = mybir.dt.float32
BF16 = mybir.dt.bfloat16
NP_BF16 = mybir.dt.np(BF16)

_PROGRAM = None  # cached (nc,) build


def _build_program():
    nc = bass.Bass()
    xt_d = nc.dram_tensor("xt", [P, KT * T_CORE], BF16, kind="ExternalInput")
    wt_d = nc.dram_tensor("wt", [P, NC_N * WSLOT], BF16, kind="ExternalInput")
    at_d = nc.dram_tensor("at", [P, KT * P], BF16, kind="ExternalInput")
    bs_d = nc.dram_tensor("bs", [P, N], BF16, kind="ExternalInput")
    out_d = nc.dram_tensor("out", [P, NC_N * TT * NCHUNK], F32, kind="ExternalOutput")

    from contextlib import ExitStack
    with ExitStack() as ctx:
        e = ctx.enter_context
        xT_s = e(nc.sbuf_tensor("xT_s", [P, KT * T_CORE], BF16))   # 64 KB/part
        w_s = e(nc.sbuf_tensor("w_s", [P, 2 * WSLOT], BF16))       # 64 KB/part
        at_s = e(nc.sbuf_tensor("at_s", [P, KT * P], BF16))        # 8 KB/part
        bs_s = e(nc.sbuf_tensor("bs_s", [P, N], BF16))             # 8 KB/part
        xaT_s = e(nc.sbuf_tensor("xaT_s", [P, T_CORE], BF16))      # 2 KB/part
        os_s = e(nc.sbuf_tensor("os_s", [P, TT * NCHUNK], F32))    # 16 KB/part
        banks = [e(nc.psum_tensor(f"pbank{i}", [P, NCHUNK], F32)) for i in range(8)]
        s_at = e(nc.semaphore("s_at"))
        xt_sems = [e(nc.semaphore(f"s_xt{i}")) for i in range(NG)]
        s_bs = e(nc.semaphore("s_bs"))
        w_sems = [e(nc.semaphore(f"s_w{i}")) for i in range(2)]
        s_pexa = e(nc.semaphore("s_pexa"))
        s_zero = e(nc.semaphore("s_zero"))
        s_xa = e(nc.semaphore("s_xa"))
        s_bank = e(nc.semaphore("s_bank"))
        s_cp = e(nc.semaphore("s_cp"))
        s_od = e(nc.semaphore("s_od"))
        block = e(nc.Block())

        def xts(k, lo, hi):
            return xT_s[:, k * T_CORE + lo:k * T_CORE + hi]

        def xtile(k, j):
            return xT_s[:, k * T_CORE + j * P:k * T_CORE + (j + 1) * P]

        def wsl(c, k):
            base = (c % 2) * WSLOT + k * NCHUNK
            return w_s[:, base:base + NCHUNK]

        def wslot_ready(c):
            # w_sems value guaranteeing chunk c resident: slot 0 gets chunk 0
            # in 4 pieces (16 each), then 2,4,6; slot 1 gets 1,3,5,7.
            if c % 2 == 0:
                return 64 + 16 * (c // 2)
            return 16 * ((c + 1) // 2)

        @block.sync
        def _(sync):
            # at head (k-tiles 0..3) + xt0 + w piece 0 are the only bytes
            # gating the first matmuls -- keep them at the front of the wire.
            ah = KG * P
            sync.dma_start(out=at_s[:, :ah], in_=at_d[:, :ah]).then_inc(s_at, 16)
            gw = KG * T_CORE
            sync.dma_start(out=xT_s[:, :gw], in_=xt_d[:, :gw]).then_inc(
                xt_sems[0], 16)
            sync.dma_start(out=at_s[:, ah:], in_=at_d[:, ah:]).then_inc(s_at, 16)
            for g in range(1, NG):
                sync.dma_start(
                    out=xT_s[:, g * gw:(g + 1) * gw],
                    in_=xt_d[:, g * gw:(g + 1) * gw],
                ).then_inc(xt_sems[g], 16)
            sync.dma_start(out=bs_s[:], in_=bs_d[:]).then_inc(s_bs, 16)

        @block.scalar
        def _(scalar):
            # w chunk stream (2-slot ring) interleaved with output stores.
            # chunk 0 in 4 pieces, gated so the startup-critical at/xT loads
            # aren't starved on the shared SDMA engines.
            for i in range(4):
                if i > 0:
                    scalar.wait_ge(xt_sems[2 * i - 1], 16)
                scalar.dma_start(
                    out=w_s[:, i * WPIECE:(i + 1) * WPIECE],
                    in_=wt_d[:, i * WPIECE:(i + 1) * WPIECE],
                ).then_inc(w_sems[0], 16)
            scalar.wait_ge(xt_sems[NG - 1], 16)
            ow = TT * NCHUNK
            hw_ = ow // 2

            def store(cc, piece):
                scalar.wait_ge(s_cp, cc * TT + 4 * (piece + 1))
                scalar.dma_start(
                    out=out_d[:, cc * ow + piece * hw_:cc * ow + (piece + 1) * hw_],
                    in_=os_s[:, piece * hw_:(piece + 1) * hw_],
                ).then_inc(s_od, 16)

            for c in range(1, NC_N):
                if c >= 2:
                    # slot's previous occupant (chunk c-2) fully drained
                    scalar.wait_ge(s_cp, (c - 1) * TT)
                scalar.dma_start(
                    out=w_s[:, (c % 2) * WSLOT:(c % 2 + 1) * WSLOT],
                    in_=wt_d[:, c * WSLOT:(c + 1) * WSLOT],
                ).then_inc(w_sems[c % 2], 16)
                if c >= 2:
                    store(c - 2, 0)
                    store(c - 2, 1)
            store(NC_N - 2, 0)
            store(NC_N - 2, 1)
            # last chunk: 4 finer pieces to shorten the tail
            qw = ow // 4
            cc = NC_N - 1
            for pq in range(4):
                scalar.wait_ge(s_cp, cc * TT + 2 * (pq + 1))
                scalar.dma_start(
                    out=out_d[:, cc * ow + pq * qw:cc * ow + (pq + 1) * qw],
                    in_=os_s[:, pq * qw:(pq + 1) * qw],
                ).then_inc(s_od, 16)

        @block.gpsimd
        def _(gpsimd):
            gpsimd.memset(xaT_s[:], 0.0).then_inc(s_zero, 1)

        @block.tensor
        def _(tensor):
            # ---- xa phase + chunk-0 base (t-tiles 2..7), interleaved with
            # xT arrival ----
            tensor.wait_ge(s_at, 16)
            for g in range(NG):
                if g == 1:
                    tensor.wait_ge(s_at, 32)  # at tail (k-tiles 4..31)
                tensor.wait_ge(xt_sems[g], 16)
                for k in range(g * KG, (g + 1) * KG):
                    a_sl = at_s[:, k * P:(k + 1) * P]
                    m0 = tensor.matmul(
                        banks[0][:], lhsT=a_sl, rhs=xts(k, 0, SEQ_LEN),
                        start=(k == 0), stop=(k == KT - 1))
                    m1 = tensor.matmul(
                        banks[1][:], lhsT=a_sl, rhs=xts(k, SEQ_LEN, T_CORE),
                        start=(k == 0), stop=(k == KT - 1))
                tensor.wait_ge(w_sems[0], 16 * (g // 2 + 1))
                for k in range(g * KG, (g + 1) * KG):
                    for j in range(2, TT):
                        tensor.matmul(
                            banks[j][:], lhsT=xtile(k, j), rhs=wsl(0, k),
                            start=(k == 0), stop=False)
            m0.then_inc(s_pexa, 1)
            m1.then_inc(s_pexa, 1)

            # chunk 0, t-tiles 0..1 (banks freed by the xaT copies), then the
            # lora closes (stops j0, j1, j2..j7 in order)
            tensor.wait_ge(s_xa, 1)
            for k in range(KT):
                tensor.matmul(banks[0][:], lhsT=xtile(k, 0), rhs=wsl(0, k),
                              start=(k == 0), stop=False)
            tensor.wait_ge(s_xa, 2)
            tensor.wait_ge(s_bs, 16)
            tensor.matmul(
                banks[0][:], lhsT=xaT_s[:, 0:P], rhs=bs_s[:, 0:NCHUNK],
                start=False, stop=True).then_inc(s_bank, 1)
            for k in range(KT):
                tensor.matmul(banks[1][:], lhsT=xtile(k, 1), rhs=wsl(0, k),
                              start=(k == 0), stop=False)
            tensor.matmul(
                banks[1][:], lhsT=xaT_s[:, P:2 * P], rhs=bs_s[:, 0:NCHUNK],
                start=False, stop=True).then_inc(s_bank, 1)
            for j in range(2, TT):
                tensor.matmul(
                    banks[j][:], lhsT=xaT_s[:, j * P:(j + 1) * P],
                    rhs=bs_s[:, 0:NCHUNK], start=False, stop=True,
                ).then_inc(s_bank, 1)

            # ---- chunks 1..7 steady state: j-outer / k-inner ----
            for c in range(1, NC_N):
                tensor.wait_ge(w_sems[c % 2], wslot_ready(c))
                for j in range(TT):
                    tensor.wait_ge(s_cp, (c - 1) * TT + j + 1)
                    tensor.matmul(
                        banks[j][:], lhsT=xaT_s[:, j * P:(j + 1) * P],
                        rhs=bs_s[:, c * NCHUNK:(c + 1) * NCHUNK],
                        start=True, stop=False)
                    for k in range(KT):
                        mm = tensor.matmul(
                            banks[j][:], lhsT=xtile(k, j), rhs=wsl(c, k),
                            start=False, stop=(k == KT - 1))
                    mm.then_inc(s_bank, 1)

        @block.vector
        def _(vector):
            # xa copies into zeroed xaT (fp32 PSUM -> bf16, valid halves only)
            vector.wait_ge(s_zero, 1)
            vector.wait_ge(s_pexa, 2)
            vector.tensor_copy(xaT_s[0:MAX_RANK, 0:SEQ_LEN],
                               banks[0][0:MAX_RANK, :]).then_inc(s_xa, 1)
            vector.tensor_copy(xaT_s[MAX_RANK:P, SEQ_LEN:T_CORE],
                               banks[1][MAX_RANK:P, :]).then_inc(s_xa, 1)
            # out copies psum -> staging
            for c in range(NC_N):
                for j in range(TT):
                    vector.wait_ge(s_bank, c * TT + j + 1)
                    if c >= 1 and j == 0:
                        vector.wait_ge(s_od, 32 * (c - 1) + 16)
                    if c >= 1 and j == 4:
                        vector.wait_ge(s_od, 32 * (c - 1) + 32)
                    vector.tensor_copy(os_s[:, j * NCHUNK:(j + 1) * NCHUNK],
                                       banks[j][:]).then_inc(s_cp, 1)

    return nc


def _get_program():
    global _PROGRAM
    if _PROGRAM is None:
        _PROGRAM = _build_program()
    return _PROGRAM


def _host_prep(x, a_cache, b_cache, base_weight, scaling,
               q_start_loc, q_seqlens, adapter_ids, rank_offset, ranks):
    """Build the 8 per-core input maps (sharding + tiny routing gathers)."""
    x = np.asarray(x, np.float32)
    a_cache = np.asarray(a_cache, np.float32)
    b_cache = np.asarray(b_cache, np.float32)
    base_weight = np.asarray(base_weight, np.float32)
    scaling = np.asarray(scaling, np.float32)
    q_start_loc = np.asarray(q_start_loc, np.int64)
    adapter_ids = np.asarray(adapter_ids, np.int64)
    rank_offset = np.asarray(rank_offset, np.int64)
    ranks = np.asarray(ranks, np.int64)

    T = x.shape[0]
    assert T == NCORES * T_CORE
    # exact reference routing: per-token adapter, then check 512-block uniformity
    tok = np.arange(T)
    seq_idx = np.searchsorted(q_start_loc, tok, side="right") - 1
    tok_adapter = adapter_ids[seq_idx]
    blocks = tok_adapter.reshape(T // SEQ_LEN, SEQ_LEN)
    assert (blocks == blocks[:, :1]).all(), "non-uniform 512-token blocks"
    block_adapter = blocks[:, 0]  # [16]

    xb = x.astype(NP_BF16)
    # wt layout: wt[p, (c*KT + k)*512 + n] = W[c*512 + n, k*128 + p]
    wb = np.ascontiguousarray(base_weight.T).astype(NP_BF16)  # [K, N]
    wt = np.ascontiguousarray(
        wb.reshape(KT, P, NC_N, NCHUNK).transpose(1, 2, 0, 3)
    ).reshape(P, NC_N * WSLOT)

    in_maps = []
    for c in range(NCORES):
        rows = slice(c * T_CORE, (c + 1) * T_CORE)
        # xt layout: xt[p, k*1024 + t] = x[row0 + t, k*128 + p]
        xt = np.ascontiguousarray(
            xb[rows].T.reshape(KT, P, T_CORE).transpose(1, 0, 2)
        ).reshape(P, KT * T_CORE)
        a_pack = np.zeros((P, K), np.float32)
        bs = np.zeros((P, N), np.float32)
        for s in range(2):  # two sequences per core
            a = int(block_adapter[2 * c + s])
            r = int(ranks[a])
            idxs = rank_offset[a, :r]
            a_pack[s * MAX_RANK: s * MAX_RANK + r, :] = a_cache[idxs]
            bs[s * MAX_RANK: s * MAX_RANK + r, :] = b_cache[idxs] * scaling[a]
        # at layout: at[p, k*128 + r] = a_pack[r, k*128 + p]
        at = np.ascontiguousarray(
            a_pack.T.astype(NP_BF16).reshape(KT, P, P).transpose(1, 0, 2)
        ).reshape(P, KT * P)
        in_maps.append({"xt": xt, "wt": wt, "at": at,
                        "bs": bs.astype(NP_BF16)})
    return in_maps


LAST_RESULT = None  # BassKernelResults of the most recent run (for profiling)


def kernel(**inputs) -> np.ndarray:
    global LAST_RESULT
    import os
    nc = _get_program()
    in_maps = _host_prep(**inputs)
    trace = os.environ.get("KERNEL_TRACE") == "1"
    kw = {}
    if trace:
        kw = dict(trace=True, trace_cores=list(range(NCORES)))
    res = run_bass_kernel_spmd(nc, in_maps, core_ids=list(range(NCORES)), **kw)
    LAST_RESULT = res
    out = np.empty((NCORES * T_CORE, N), np.float32)
    for c in range(NCORES):
        # out buf: [p, (cc*TT + j)*512 + n] -> out[j*128 + p, cc*512 + n]
        buf = res.results[c]["out"].reshape(P, NC_N, TT, NCHUNK)
        out[c * T_CORE:(c + 1) * T_CORE] = (
            buf.transpose(2, 0, 1, 3).reshape(T_CORE, N))
    return out
